# revision 24
# baseline (speedup 1.0000x reference)
"""Trainium2 Bass kernel for pairwise radial-angular graph convolution.

Computes, for z in 0..3 (batch), a,b in 0..511 (points), i,j in 0..15:
  rel = g[z,b] - g[z,a];  d = sqrt(|rel|^2 + eps)
  rad_r = exp(-gamma*(d - c_r)^2)          (8 radial shells)
  ang   = [1, rel/d]                        (4 angular fns)
  out[z,a,i] = 1/sqrt(n) * sum_{b,r,m,j} rad_r*ang_m*W[r,m,i,j]*f[z,b,j]

v3 structure (per core: one z, one 256-wide a-half; b full 512):
  nd2[b,a] = -gamma*d^2            rank-6 factored matmul (PE, K=6, f32r)
  clamp nd2 <= 0 (DVE min, hoisted into the early idle window)
  lt  = ln(-nd2/g + dmin^2)        ACT set 6; dmin blends a floor into d
                                   (replaces the baseline's dmask DMA + mul:
                                   self pairs get d=dmin, 1/d=1/dmin; their
                                   V-terms cancel between the Vb matmul and
                                   the -g_a*V correction)
  d   = exp(lt/2); sq7 = (d-c7)^2; E7 = exp(-g*sq7 + g*c7^2)  (anchor)
  minv = exp(-2g*dc*d);  rcp = exp(-lt/2)                     (all ACT)
  E_r = E_{r+1}*minv               single-step downward cascade (DVE bf16)
  Q_r = E_r*rcp                    independent muls (DVE pairs + Pool)
  Contraction: pair tensors E_r/Q_r are the matmul STATIONARY [128b x 128a];
  movers are small bf16 weight-column blocks (host f*W combos, hi+lo bf16
  pair, 160 cols per (bt, shell)).  PSUM pre-zeroed, all matmuls accumulate.
  out[a,i] = psum_S + psum_Vb - sum_c g_ac*psum_Vc   (DVE final combine)
  Pipeline: column-chunked; ACT chain per chunk feeds DVE cascade feeds PE.

Sharding: 8 cores = 4 z x 2 a-halves; full output gathered on host.
"""

import math

import numpy as np

# ---------------------------------------------------------------- constants
Z, NPTS, C_IN, C_OUT = 4, 512, 16, 16
NUM_RADIAL, NUM_ANGULAR = 8, 4
MAX_R, GAMMA = 3.0, 8.0
N_CORES = 8
A_PER_CORE = NPTS // 2          # 256 output points per core
N_BT = NPTS // 128              # 4 b-tiles of 128
PAIRC = N_BT * A_PER_CORE       # 1024 free cols of pair tensors
CENTERS = [MAX_R * r / (NUM_RADIAL - 1) for r in range(NUM_RADIAL)]
DC = CENTERS[1] - CENTERS[0]    # shell spacing 3/7
C7 = CENTERS[7]
C7SQ = GAMMA * C7 * C7          # exp-arg offset for the anchor shell
DMIN = 0.02                     # soft floor on d (self pairs / eps)
GWB = 160                       # weight cols per (bt, shell): hi80 | lo80

# knobs
import os
CHUNKS = tuple(int(x) for x in os.environ.get(
    "K_CHUNKS", "128,256,256,384").split(","))
POOL_Q = tuple(int(x) for x in os.environ.get(
    "K_POOLQ", "5,3,1").split(",") if x)
POOL_Q_LAST = tuple(int(x) for x in os.environ.get(
    "K_POOLQL", "5,3").split(",") if x)
POOL_CLAMP = tuple(int(x) for x in os.environ.get(
    "K_POOLCL", "").split(",") if x)  # b-tiles whose clamp runs on GPSIMD
assert sum(CHUNKS) == PAIRC and all(c % 128 == 0 for c in CHUNKS)
assert 0 not in POOL_Q_LAST     # shell 0 carries the psum stop flag

_CACHE = {}


def _build_program():
    import concourse.bacc as bacc
    import concourse.mybir as mybir
    import concourse.tile as tile

    f32 = mybir.dt.float32
    f32s = mybir.dt.float32r
    bf16 = mybir.dt.bfloat16
    AF = mybir.ActivationFunctionType

    nc = bacc.Bacc("TRN2", target_bir_lowering=False, debug=False)

    # activation-bias constants: allocate up front, memset inside the
    # TileContext so the dependency tracker orders them against their ACT
    # readers
    const_tiles = []
    for v in (DMIN * DMIN, C7SQ, -C7):
        t = nc.alloc_sbuf_tensor(f"const-f32-{v}", [128, 1], f32)
        nc.const_aps.aps[(f32, v)] = t.ap()
        const_tiles.append((t, v))

    # ---------------- IO -------------------------------------------------
    ba_d = nc.dram_tensor("ba", [6, NPTS + A_PER_CORE], f32s,
                          kind="ExternalInput")
    gat_d = nc.dram_tensor("gat", [128, 6], f32, kind="ExternalInput")
    gw_d = nc.dram_tensor("gw", [128, N_BT * NUM_RADIAL * GWB], bf16,
                          kind="ExternalInput")
    out_d = nc.dram_tensor("out", [128, 32], f32, kind="ExternalOutput")

    A = A_PER_CORE

    with tile.TileContext(nc) as tc:
        with (
            tc.tile_pool(name="const", bufs=1) as cpool,
            tc.tile_pool(name="work", bufs=1) as wpool,
            tc.tile_pool(name="ndp", bufs=1, space="PSUM") as ndpool,
            tc.tile_pool(name="acc", bufs=1, space="PSUM") as accpool,
            tc.tile_pool(name="fin", bufs=1) as fpool,
        ):
            # ---------------- inputs ------------------------------------
            ba = cpool.tile([6, NPTS + A], f32s, tag="ba")
            gat = cpool.tile([128, 6], f32, tag="gat")
            gw = cpool.tile([128, N_BT * NUM_RADIAL * GWB], bf16, tag="gw")
            nc.sync.dma_start(out=ba[:], in_=ba_d.ap())
            for t, v in const_tiles:
                nc.gpsimd.memset(t.ap(), v)
            # gw split by b-tile so the first chunks' weights land early
            BTW = NUM_RADIAL * GWB
            nc.sync.dma_start(out=gw[:, 0:BTW], in_=gw_d.ap()[:, 0:BTW])
            nc.sync.dma_start(out=gw[:, BTW:2 * BTW],
                              in_=gw_d.ap()[:, BTW:2 * BTW])
            nc.sync.dma_start(out=gw[:, 2 * BTW:4 * BTW],
                              in_=gw_d.ap()[:, 2 * BTW:4 * BTW])
            nc.sync.dma_start(out=gat[:], in_=gat_d.ap())
            b6 = ba[:, 0:NPTS]
            a6 = ba[:, NPTS:NPTS + A]

            # single ln+exp table set, loaded once up front
            preload = mybir.InstLoadActFuncSet(
                name=nc.get_next_instruction_name(),
                act_func_set_id=6, ins=[], outs=[])
            preload.engine = mybir.EngineType.Activation
            nc.scalar.add_instruction(preload)

            # ---------------- pair-tensor buffers -----------------------
            # one psum tile per b-tile so mm(bt+1) doesn't serialize behind
            # clamp(bt) through tile-granularity dependency tracking
            ndps = []
            for bt in range(N_BT):
                ndp = ndpool.tile([128, A], f32, tag=f"ndps{bt}")
                ndps.append(ndp)
            nd2 = wpool.tile([128, PAIRC], f32, tag="nd2")
            lt = wpool.tile([128, PAIRC], f32, tag="lt")
            dd = wpool.tile([128, PAIRC], f32, tag="dd")
            sq7 = wpool.tile([128, PAIRC], f32, tag="sq7")
            minv = wpool.tile([128, PAIRC], bf16, tag="minv")
            rcp = wpool.tile([128, PAIRC], bf16, tag="rcp")
            Eb = wpool.tile([128, NUM_RADIAL * PAIRC], bf16, tag="Eb")
            Qb = wpool.tile([128, NUM_RADIAL * PAIRC], bf16, tag="Qb")

            def shl(big, r, cs):
                return big[:, r * PAIRC + cs.start:r * PAIRC + cs.stop]

            def shl3(big, r0, r1, cs):        # [128, nr, cw] strided view
                return big.rearrange(
                    "p (r q) -> p r q", r=NUM_RADIAL)[:, r0:r1,
                                                      cs.start:cs.stop]

            def bcast3(t, cs, n):             # [128, cw] -> [128, n, cw]
                return t[:, cs].rearrange(
                    "p (o q) -> p o q", o=1).to_broadcast(
                        [128, n, cs.stop - cs.start])

            # merged psum: per blk 64 cols = [S+Vb accumulated 16 | V 48].
            # The Q matmul's mover is [Vb16|V48]; its dst starts at the S
            # region so Vb accumulates straight onto S, killing one op in
            # the final combine.
            pc = accpool.tile([128, 128], f32, tag="pc")
            nc.vector.memset(pc[:], 0.0)

            def contraction(big, r, bt, blk, v, last=False):
                rd = NUM_RADIAL - 1 - r
                gbase = bt * (NUM_RADIAL * GWB) + rd * GWB + (16 if v else 0)
                gn = 64 if v else 16
                dst = pc[:, blk * 64:blk * 64 + gn]
                ssl = slice(r * PAIRC + bt * A + blk * 128,
                            r * PAIRC + bt * A + blk * 128 + 128)
                # bf16x2: hi + lo weight halves accumulate into one psum
                nc.tensor.matmul(
                    dst, big[:, ssl], gw[:, gbase:gbase + gn],
                    start=False, stop=False, skip_group_check=True)
                nc.tensor.matmul(
                    dst, big[:, ssl], gw[:, 80 + gbase:80 + gbase + gn],
                    start=False, stop=last, skip_group_check=True)

            # all -gamma*d^2 matmuls + clamps up front: clamps run during
            # the early idle window (DVE for tile 0, GPSIMD for the rest).
            # bt0's clamp is split so chunk 0 (its first CHUNKS[0] cols)
            # unblocks the ACT chain as early as possible.
            # (GPSIMD cannot read PSUM, so all clamps stay on DVE — they
            # run in its long idle window before the first cascade mul)
            c0 = min(CHUNKS[0], A)
            for bt in range(N_BT):
                nc.tensor.matmul(
                    ndps[bt][:], b6[:, bt * 128:(bt + 1) * 128], a6,
                    start=True, stop=True)
                if bt == 0 and c0 < A:
                    with tc.high_priority():
                        nc.vector.tensor_scalar_min(
                            nd2[:, 0:c0], ndps[0][:, 0:c0], 0.0)
                    nc.vector.tensor_scalar_min(nd2[:, c0:A],
                                                ndps[0][:, c0:A], 0.0)
                else:
                    bs = slice(bt * A, (bt + 1) * A)
                    nc.vector.tensor_scalar_min(nd2[:, bs], ndps[bt][:], 0.0)

            # chunk column ranges; each 128-col block maps to (bt, blk)
            def blocks(cs):
                out = []
                for col in range(cs.start, cs.stop, 128):
                    out.append((col // A, (col % A) // 128))
                return out

            t0 = 0
            nch = len(CHUNKS)
            for ci, cw in enumerate(CHUNKS):
                cs = slice(t0, t0 + cw)
                t0 += cw
                lastch = ci == nch - 1
                blks = blocks(cs)

                # ---- ACT chain (set 6: ln + exp + square) ----
                # rcp right after Ln so Q7 (needs E7+rcp) unblocks at pass 5.
                # chunk 0's chain is the pipeline-fill critical path: pin it
                # ahead of everything in the scheduler's priority heap.
                import contextlib
                prio = tc.high_priority() if ci == 0 else \
                    contextlib.nullcontext()
                with prio:
                    nc.scalar.activation(lt[:, cs], nd2[:, cs], AF.Ln,
                                         bias=DMIN * DMIN,
                                         scale=-1.0 / GAMMA)
                    nc.scalar.activation(rcp[:, cs], lt[:, cs], AF.Exp,
                                         scale=-0.5)
                    nc.scalar.activation(dd[:, cs], lt[:, cs], AF.Exp,
                                         scale=0.5)
                    nc.scalar.activation(sq7[:, cs], dd[:, cs], AF.Square,
                                         bias=-C7)
                    nc.scalar.activation(shl(Eb, 7, cs), sq7[:, cs], AF.Exp,
                                         bias=C7SQ, scale=-GAMMA)
                    nc.scalar.activation(minv[:, cs], dd[:, cs], AF.Exp,
                                         scale=-2.0 * GAMMA * DC)

                # ---- E cascade (single-step) + Q muls + contractions ----
                # Contraction emission follows production order (PE.SEQ is
                # in-order: a stalled Ldweights blocks everything behind
                # it).  POOL_Q shells' Q-muls run on slow GPSIMD right
                # after their E exists, but contract only at chunk end
                # when they are long done.
                poolq = POOL_Q_LAST if lastch else POOL_Q
                for bt, blk in blks:
                    contraction(Eb, 7, bt, blk, False)
                # Q7 first: needs only E7 + rcp, unblocks DVE at ACT pass 5
                if 7 in poolq:
                    nc.gpsimd.tensor_mul(shl(Qb, 7, cs), shl(Eb, 7, cs),
                                         rcp[:, cs])
                else:
                    nc.vector.tensor_mul(shl(Qb, 7, cs), shl(Eb, 7, cs),
                                         rcp[:, cs])
                    for bt, blk in blks:
                        contraction(Qb, 7, bt, blk, True)

                # DVE Q-pair rounds: in steady chunks, delay the pair one
                # cascade round so its inputs are old news and the op can
                # fill the sem-gap after the chained E-mul; in the last
                # chunk keep pairs at their earliest round for a short tail
                def qpair(r0, r1):
                    dv = [q for q in range(r0, r1) if q not in poolq]
                    if len(dv) == 2:
                        nc.vector.tensor_mul(shl3(Qb, r0, r0 + 2, cs),
                                             shl3(Eb, r0, r0 + 2, cs),
                                             bcast3(rcp, cs, 2))
                    else:
                        for q in dv:
                            nc.vector.tensor_mul(shl(Qb, q, cs),
                                                 shl(Eb, q, cs), rcp[:, cs])
                    return sorted(dv, reverse=True)

                for r in range(6, -1, -1):
                    nc.vector.tensor_mul(shl(Eb, r, cs), shl(Eb, r + 1, cs),
                                         minv[:, cs])
                    if r in poolq:
                        # pooled shell: launch as soon as E_r exists
                        nc.gpsimd.tensor_mul(shl(Qb, r, cs),
                                             shl(Eb, r, cs), rcp[:, cs])
                    qnow = []
                    if lastch:
                        if r % 2 == 1 or r == 0:
                            qnow = qpair(r, r + 2 if r % 2 == 1 else r + 1)
                    elif r % 2 == 0:
                        qnow = qpair(r + 1, r + 3)
                    for bt, blk in blks:
                        contraction(Eb, r, bt, blk, False)
                    for q in qnow:
                        for bt, blk in blks:
                            contraction(Qb, q, bt, blk, True,
                                        last=(lastch and q == 0))
                    if r == 1:
                        # pooled shells: produced early on GPSIMD, long done
                        # by now; contract before the final r=0 round
                        for q in sorted(poolq, reverse=True):
                            for bt, blk in blks:
                                contraction(Qb, q, bt, blk, True)
                if not lastch:
                    # trailing round for the delayed pairing: shell 0
                    for q in qpair(0, 1):
                        for bt, blk in blks:
                            contraction(Qb, q, bt, blk, True)

            # ---------------- final combine -----------------------------
            # out = (S + Vb) - sum_c g_ac V_c  (gat holds -g_a); S+Vb is
            # already accumulated in psum; ops span both a-blocks at once
            osb = fpool.tile([128, 32], f32, tag="osb")
            w3 = fpool.tile([128, 96], f32, tag="w3")
            tmp = fpool.tile([128, 32], f32, tag="tmp")
            pcv = pc.rearrange("p (blk x) -> p blk x", blk=2)
            # w3 laid out c-innermost so one reduce-X sums over c
            w3v = w3.rearrange("p (blk i c) -> p blk i c", blk=2, i=16)
            nc.vector.tensor_mul(
                w3v, pcv[:, :, 16:64].rearrange("p blk (c i) -> p blk i c",
                                                c=3),
                gat.rearrange("p (blk o c) -> p blk o c", blk=2,
                              o=1).to_broadcast([128, 2, 16, 3]))
            nc.vector.tensor_reduce(
                tmp.rearrange("p (blk i) -> p blk i", blk=2), w3v,
                mybir.AxisListType.X, mybir.AluOpType.add)
            nc.vector.tensor_tensor(
                osb.rearrange("p (blk i) -> p blk i", blk=2),
                pcv[:, :, 0:16], tmp.rearrange("p (blk i) -> p blk i",
                                               blk=2), mybir.AluOpType.add)
            nc.sync.dma_start(out=out_d.ap(), in_=osb[:])

    nc.compile()
    return nc


def _host_prep(features, geometry, W, n_norm):
    """Build per-core input maps (all small host-side tensors)."""
    import ml_dtypes

    f = np.asarray(features, dtype=np.float32)
    g = np.asarray(geometry, dtype=np.float32)
    W = np.asarray(W, dtype=np.float32)
    scale = 1.0 / math.sqrt(float(n_norm))

    # fold 1/sqrt(n) and exp(-gamma c_r^2) (cascade anchor fold) into W
    Wp = W.astype(np.float64) * scale
    for r in range(NUM_RADIAL):
        Wp[r] *= math.exp(-GAMMA * CENTERS[r] ** 2)

    in_maps = []
    for core in range(N_CORES):
        z, half = core // 2, core % 2
        gz = g[z]                                    # [512, 3]
        fz = f[z]                                    # [512, 16]
        a0 = half * A_PER_CORE
        ga = gz[a0:a0 + A_PER_CORE]                  # [256, 3]

        ba = np.empty((6, NPTS + A_PER_CORE), dtype=np.float32)
        ba[0:3, :NPTS] = gz.T
        ba[3, :NPTS] = (gz * gz).sum(axis=1)
        ba[4, :NPTS] = 1.0
        ba[5, :NPTS] = 0.0
        ba[0:3, NPTS:] = 2.0 * GAMMA * ga.T
        ba[3, NPTS:] = -GAMMA
        ba[4, NPTS:] = -GAMMA * (ga * ga).sum(axis=1)
        ba[5, NPTS:] = 1.0

        # gat[p, 3*blk + c] = -g_a for a = a0 + 128*blk + p
        gat = np.empty((128, 6), dtype=np.float32)
        for blk in range(2):
            gat[:, 3 * blk:3 * blk + 3] = -ga[blk * 128:(blk + 1) * 128]

        # gw[b-part, bt*1280 + rd*160 + [hi80 | lo80]], rd = 7-r
        # hi80/lo80 = [S16 | Vb16 | V48] bf16 hi/lo split
        # S_r[b,i]  = sum_j Wp[r,0,i,j] f[b,j]
        # Vb_r[b,i] = sum_cj g[b,c] Wp[r,c+1,i,j] f[b,j]
        # V_rc[b,i] = sum_j Wp[r,c+1,i,j] f[b,j]
        S = np.einsum('rij,bj->bri', Wp[:, 0], fz.astype(np.float64))
        V = np.einsum('rcij,bj->brci', Wp[:, 1:], fz.astype(np.float64))
        Vb = np.einsum('bc,brci->bri', gz.astype(np.float64), V)
        gwf = np.empty((NPTS, NUM_RADIAL, 80), dtype=np.float64)
        gwf[:, :, 0:16] = S
        gwf[:, :, 16:32] = Vb
        gwf[:, :, 32:80] = V.reshape(NPTS, NUM_RADIAL, 48)
        gwf = gwf[:, ::-1, :]                        # rd = 7-r ordering
        gwh = gwf.astype(ml_dtypes.bfloat16)
        gwl = (gwf - gwh.astype(np.float64)).astype(ml_dtypes.bfloat16)
        gwx = np.empty((NPTS, NUM_RADIAL, GWB), dtype=ml_dtypes.bfloat16)
        gwx[:, :, 0:80] = gwh
        gwx[:, :, 80:160] = gwl
        gw = np.ascontiguousarray(
            gwx.reshape(N_BT, 128, NUM_RADIAL * GWB)
               .transpose(1, 0, 2).reshape(128, N_BT * NUM_RADIAL * GWB))

        in_maps.append({"ba": ba, "gat": gat, "gw": gw})
    return in_maps


def kernel(features, geometry, W, n_norm):
    from concourse.bass_utils import run_bass_kernel_spmd

    if "nc" not in _CACHE:
        _CACHE["nc"] = _build_program()
    nc = _CACHE["nc"]

    in_maps = _host_prep(features, geometry, W, n_norm)
    res = run_bass_kernel_spmd(nc, in_maps, list(range(N_CORES)))

    out = np.empty((Z, NPTS, C_OUT), dtype=np.float32)
    for core in range(N_CORES):
        z, half = core // 2, core % 2
        o = res.results[core]["out"]                 # [128, 32]
        a0 = half * A_PER_CORE
        for blk in range(2):
            out[z, a0 + blk * 128:a0 + (blk + 1) * 128, :] = \
                o[:, blk * 16:(blk + 1) * 16]
    return out
